# revision 1
# baseline (speedup 1.0000x reference)
"""Trainium2 Bass kernel for nn_BestHits: out = bh * bh.T where
bh = blockwise-softmax(mask_diag(similarities) / TAU) over 256-wide column groups.

Strategy: out is symmetric (out.T = bh.T * bh = out), so only the upper
triangle of 512x512 block-pairs is computed on device. The 16x16 block grid
has 136 upper-incl-diagonal pairs = 17 per core on 8 cores (each core gets
exactly 2 diagonal + 15 off-diagonal pairs -> perfectly uniform SPMD work).

Per pair (I, J): A = sims[I, J]-block, B = sims[J, I]-block (diagonal
pre-masked on host for I == J). Softmax groups (256 cols) are block-aligned,
so each factor's normalization is fully local to its block:
    bhA = gsoftmax(A), bhB = gsoftmax(B), out[I, J] = bhA * bhB.T
and out[J, I] = out[I, J].T is mirrored on the host. Diagonal pairs have
B == A, so the last 2 slots of every core load only A and reuse its exp/sums.

Device pipeline per slot (engines balanced under the DMA roofline):
  - ACT: zA = exp(A/TAU) as one big-FD activation; zB = exp(B/TAU) as 8
    per-group activations with accum_out giving zB's group sums for free.
  - DVE: zA group sums via one 3D tensor_reduce; reciprocals; 8 tiny
    tensor_scalar ops build D = diag(rb) tiles from the identity.
  - PE:  bhB.T = zB.T @ D via regular fp32 matmuls (the diagonal folds the
    1/sum scaling into the transpose, and regular matmuls keep the PE's
    HAM clock warm, unlike transpose-mode).
  - DVE: out = (zA * ra) * bhB.T fused by scalar_tensor_tensor from PSUM.

Per-core HBM traffic: 2*2 MiB (diag) + 15*3 MiB = 49 MiB -> ~143 us
roofline at ~358 GB/s.
"""
import sys

import numpy as np

sys.path.insert(0, "/opt/trn_rl_repo")

from contextlib import ExitStack

import concourse.bass as bass  # noqa: F401  (registers AP machinery)
import concourse.tile as tile
from concourse import bacc, masks, mybir
from concourse.bass_utils import run_bass_kernel_spmd

N = 8192          # full matrix side
B = 512           # block side
NB = N // B       # 16 blocks per side
P = 128           # SBUF partitions
T = B // P        # 4 row-subtiles per block
GRP = 256         # softmax group width
NG = B // GRP     # 2 groups per block side
TAU = 0.1
NDIAG = 2         # diagonal pairs per core (the last NDIAG slots)
NSLOTS = 17       # block-pairs per core
NCORES = 8
MASK = -1e30      # pre-masked diagonal value (exp(MASK/TAU) == 0 in f32)

F32 = mybir.dt.float32
AF = mybir.ActivationFunctionType
OP = mybir.AluOpType


def core_pairs() -> list[list[tuple[int, int]]]:
    """136 upper-triangle block pairs distributed 17-per-core; the 2 diagonal
    pairs of each core come last (the kernel treats those slots specially)."""
    diag = [(i, i) for i in range(NB)]
    off = [(i, j) for i in range(NB) for j in range(i + 1, NB)]
    cps: list[list[tuple[int, int]]] = [[] for _ in range(NCORES)]
    for idx, p in enumerate(off):
        cps[idx % NCORES].append(p)
    for idx, p in enumerate(diag):
        cps[idx % NCORES].append(p)
    return cps


CORE_PAIRS = core_pairs()


def build():
    """Build + compile the (single-program, 8-core SPMD) Bass kernel."""
    nc = bacc.Bacc(
        "TRN2",
        target_bir_lowering=False,
        debug=False,
        enable_asserts=True,
        num_devices=NCORES,
    )
    a = nc.dram_tensor("a", [NSLOTS, P, T, B], F32, kind="ExternalInput").ap()
    b = nc.dram_tensor("b", [NSLOTS - NDIAG, P, T, B], F32, kind="ExternalInput").ap()
    o = nc.dram_tensor("o", [NSLOTS, P, T, B], F32, kind="ExternalOutput").ap()

    with tile.TileContext(nc) as tc, ExitStack() as ctx:
        const_pool = ctx.enter_context(tc.tile_pool(name="const", bufs=1))
        ident = const_pool.tile([P, P], F32)
        masks.make_identity(nc, ident[:])

        a_pool = ctx.enter_context(tc.tile_pool(name="a_sb", bufs=4))
        b_pool = ctx.enter_context(tc.tile_pool(name="b_sb", bufs=4))
        za_pool = ctx.enter_context(tc.tile_pool(name="za", bufs=3))
        zb_pool = ctx.enter_context(tc.tile_pool(name="zb", bufs=3))
        o_pool = ctx.enter_context(tc.tile_pool(name="o_sb", bufs=3))
        st_pool = ctx.enter_context(tc.tile_pool(name="st", bufs=8))
        d_pool = ctx.enter_context(tc.tile_pool(name="dg", bufs=3))
        ps_pool = ctx.enter_context(tc.tile_pool(name="ps", bufs=2, space="PSUM"))

        for k in range(NSLOTS):
            diag_slot = k >= NSLOTS - NDIAG
            if not diag_slot:
                b_sb = b_pool.tile([P, T, B], F32)
                nc.sync.dma_start(b_sb[:], b[k])
            # Block rows fold into (subtile t, partition p): row r = t*P + p.
            a_sb = a_pool.tile([P, T, B], F32)
            nc.sync.dma_start(a_sb[:], a[k])

            # zA = exp(A/TAU), one big-FD activation; group sums on DVE.
            za = za_pool.tile([P, T, B], F32)
            nc.scalar.activation(za[:], a_sb[:], AF.Exp, scale=1.0 / TAU)
            sa = st_pool.tile([P, T, NG], F32)
            nc.vector.tensor_reduce(
                sa[:],
                za[:].rearrange("p t b -> p (t b)").rearrange(
                    "p (G s) -> p G s", s=GRP
                ),
                axis=mybir.AxisListType.X,
                op=OP.add,
            )
            ra = st_pool.tile([P, T, NG], F32)
            nc.vector.reciprocal(ra[:], sa[:])

            # Four PSUM tiles (one bank each) hold bhB.T's row-subtiles.
            pss = [ps_pool.tile([P, B], F32, name=f"ps{v}") for v in range(T)]
            o_sb = o_pool.tile([P, T, B], F32)

            if diag_slot:
                zb, rb = za, ra
                dg = d_pool.tile([P, T * NG, P], F32)
                nc.vector.tensor_mul(
                    dg[:],
                    ident[:].rearrange("p (one c) -> p one c", one=1)
                    .broadcast_to([P, T * NG, P]),
                    rb[:].rearrange("p t (g one) -> p (t g) one", one=1)
                    .broadcast_to([P, T * NG, P]),
                )
                for u in range(T):
                    for v in range(T):
                        nc.tensor.matmul(
                            pss[v][:, u * P:(u + 1) * P],
                            zb[:, u, v * P:(v + 1) * P],
                            dg[:, u * NG + (v // 2), :],
                        )
            else:
                # Per-u pipeline: as soon as subtile u's two group-exps (with
                # accumulated sums) land, build its D = diag(1/sums) tiles and
                # issue its four transpose-and-scale matmuls. The PE starts
                # ~5 us earlier per slot than with a whole-block barrier.
                zb = zb_pool.tile([P, T, B], F32)
                dg = d_pool.tile([P, T * NG, P], F32)
                for u in range(T):
                    sb = st_pool.tile([P, NG], F32, name=f"sb{u}")
                    for g in range(NG):
                        cs = slice(g * GRP, (g + 1) * GRP)
                        nc.scalar.activation(
                            zb[:, u, cs], b_sb[:, u, cs], AF.Exp,
                            scale=1.0 / TAU, accum_out=sb[:, g:g + 1],
                        )
                    rb = st_pool.tile([P, NG], F32, name=f"rb{u}")
                    nc.vector.reciprocal(rb[:], sb[:])
                    nc.vector.tensor_mul(
                        dg[:, u * NG:(u + 1) * NG, :],
                        ident[:].rearrange("p (one c) -> p one c", one=1)
                        .broadcast_to([P, NG, P]),
                        rb[:].rearrange("p (g one) -> p g one", one=1)
                        .broadcast_to([P, NG, P]),
                    )
                    for v in range(T):
                        nc.tensor.matmul(
                            pss[v][:, u * P:(u + 1) * P],
                            zb[:, u, v * P:(v + 1) * P],
                            dg[:, u * NG + (v // 2), :],
                        )

            # Fused product out[:, v, :] = (za[:, v, :] * ra) * bhB.T, then
            # per-subtile stores that overlap the remaining compute.
            for v in range(T):
                for g in range(NG):
                    cs = slice(g * GRP, (g + 1) * GRP)
                    nc.vector.scalar_tensor_tensor(
                        o_sb[:, v, cs], za[:, v, cs], ra[:, v, g:g + 1],
                        pss[v][:, cs], op0=OP.mult, op1=OP.mult,
                    )
            # One whole-block store per slot on the SWDGE (gpsimd) ring: it
            # never queues ahead of loads on the sync HWDGE ring, and a
            # single 1 MiB transfer keeps descriptor overhead amortized.
            nc.gpsimd.dma_start(o[k], o_sb[:])

    nc.compile()
    return nc


_NC = None


def _get_nc():
    global _NC
    if _NC is None:
        _NC = build()
    return _NC


def _to_pmajor(blocks: np.ndarray) -> np.ndarray:
    # (n, 512, 512) row-major -> (n, 128, 4, 512): row r = t*P + p lands at
    # [p, t, :], so every SBUF partition's bytes are contiguous in DRAM and
    # each load is 128 clean 8 KiB descriptor runs instead of 512 x 2 KiB.
    n = blocks.shape[0]
    return np.ascontiguousarray(
        blocks.reshape(n, T, P, B).transpose(0, 2, 1, 3)
    )


def make_in_maps(sims: np.ndarray) -> list[dict[str, np.ndarray]]:
    in_maps = []
    for c in range(NCORES):
        a_stack = np.empty((NSLOTS, B, B), np.float32)
        b_stack = np.empty((NSLOTS - NDIAG, B, B), np.float32)
        for k, (i, j) in enumerate(CORE_PAIRS[c]):
            a_stack[k] = sims[i * B:(i + 1) * B, j * B:(j + 1) * B]
            if i == j:
                assert k >= NSLOTS - NDIAG
                np.fill_diagonal(a_stack[k], MASK)
            else:
                b_stack[k] = sims[j * B:(j + 1) * B, i * B:(i + 1) * B]
        in_maps.append({"a": _to_pmajor(a_stack), "b": _to_pmajor(b_stack)})
    return in_maps


def assemble(results: list[dict[str, np.ndarray]]) -> np.ndarray:
    out = np.empty((N, N), np.float32)
    for c in range(NCORES):
        o_pm = results[c]["o"]  # (NSLOTS, P, T, B) partition-major
        o_stack = np.ascontiguousarray(
            o_pm.transpose(0, 2, 1, 3).reshape(NSLOTS, B, B)
        )
        for k, (i, j) in enumerate(CORE_PAIRS[c]):
            out[i * B:(i + 1) * B, j * B:(j + 1) * B] = o_stack[k]
            if i != j:
                out[j * B:(j + 1) * B, i * B:(i + 1) * B] = o_stack[k].T
    return out


def run_on_hw(sims: np.ndarray, **spmd_kwargs):
    """Run the kernel on the 8 NeuronCores. Returns (out, BassKernelResults).

    The device occasionally throws a transient NRT_EXEC_UNIT_UNRECOVERABLE
    and needs ~a minute to come back, so failed runs are retried."""
    import time

    nc = _get_nc()
    in_maps = make_in_maps(sims)
    last_exc = None
    for attempt in range(3):
        if attempt:
            time.sleep(75)
        try:
            res = run_bass_kernel_spmd(
                nc, in_maps, core_ids=list(range(NCORES)), **spmd_kwargs
            )
            return assemble(res.results), res
        except Exception as exc:  # noqa: BLE001 - device flake, retry
            last_exc = exc
    raise last_exc


def kernel(similarities: np.ndarray) -> np.ndarray:
    sims = np.ascontiguousarray(similarities, dtype=np.float32)
    assert sims.shape == (N, N)
    out, _ = run_on_hw(sims)
    return out


if __name__ == "__main__":
    rng = np.random.default_rng(0)
    sims = rng.standard_normal((N, N), dtype=np.float32)
    out = kernel(similarities=sims)
    print("out", out.shape, out.dtype, float(out.max()))

